# revision 4
# baseline (speedup 1.0000x reference)
"""GAT 2-layer GNN on 8 Trainium2 NeuronCores (Bass/Tile). Dev module (v2).

Sharding: nodes partitioned contiguously across 8 cores by dst ownership.
Per core, owned nodes are degree-sorted ((max,min) of lo/hi in-degrees) and
processed in 128-node groups: node-on-partition layout, incoming-edge slots
along the free axis. Slots [0,L) gather from the low table half, [L,L+H)
from the high half (int16 dma_gather index limit). Layer-1 per-head feature
blocks live in a rotated basis whose first two coordinates are el/er, so one
1024-byte gather row per edge carries features + logits; the aggregate is
rotated back by a block-diagonal matmul on the TensorEngine. The layer-1 ->
layer-2 halo exchange is a single AllGather collective.
"""
import sys

sys.path.insert(0, "/opt/trn_rl_repo")
import numpy as np

import concourse.bass as bass
import concourse.bacc as bacc
import concourse.mybir as mybir
import concourse.tile as tile
from concourse.bass_utils import run_bass_kernel_spmd
from concourse.masks import make_identity

F32 = mybir.dt.float32
I16 = mybir.dt.int16

NC = 8
P = 128
NEG = 0.2
CMAX = 40  # max slots per compute chunk


class Cfg:
    def __init__(self, N, E, IN=256, H1=4, D1=64, O2=47):
        assert N % NC == 0
        self.N, self.E, self.IN, self.H1, self.D1, self.O2 = N, E, IN, H1, D1, O2
        assert H1 * D1 == IN
        self.NPC = N // NC
        self.G = -(-self.NPC // P)
        self.NPAD = self.G * P
        self.HALF_NODE = N // 2
        self.ROW_HALF = (NC // 2) * self.NPAD
        self.C2 = 64


class Sched:
    pass


def preprocess(cfg, x, src, dst):
    """Per-core shards + shared schedule. dst must be sorted. The index blob
    is a single [128, 8*SUMD] int16 array whose column offset for slot d of
    group g is 8*(Doff[g]+d) — identical to the mask column layout."""
    c = cfg
    e0 = np.searchsorted(dst, np.arange(NC + 1) * c.NPC).astype(np.int64)

    l1row = np.empty(c.N, np.int64)
    core_data = []
    Ls = np.zeros((NC, c.G), np.int64)
    Hs = np.zeros((NC, c.G), np.int64)
    for ci in range(NC):
        sl = slice(e0[ci], e0[ci + 1])
        ln = (dst[sl] - ci * c.NPC).astype(np.int64)
        s = src[sl].astype(np.int64)
        hi = s >= c.HALF_NODE
        lo_deg = np.bincount(ln[~hi], minlength=c.NPC)
        hi_deg = np.bincount(ln[hi], minlength=c.NPC)
        order = np.lexsort(
            (np.minimum(lo_deg, hi_deg), np.maximum(lo_deg, hi_deg))
        )
        l1row[ci * c.NPC + order] = ci * c.NPAD + np.arange(c.NPC)
        lo_pad = np.zeros(c.NPAD, np.int64)
        hi_pad = np.zeros(c.NPAD, np.int64)
        lo_pad[: c.NPC] = lo_deg[order]
        hi_pad[: c.NPC] = hi_deg[order]
        Ls[ci] = lo_pad.reshape(c.G, P).max(1)
        Hs[ci] = hi_pad.reshape(c.G, P).max(1)
        core_data.append((ln, s, hi, order, lo_pad, hi_pad))

    S = Sched()
    S.L = Ls.max(0)
    S.H = Hs.max(0)
    S.D = S.L + S.H
    S.Doff = np.concatenate([[0], np.cumsum(S.D)])
    S.SUMD = int(S.D.sum())
    S.group_chunks = []
    for g in range(c.G):
        D = int(S.D[g])
        lst = []
        nch = max(1, -(-D // CMAX))
        base, rem = divmod(D, nch) if nch else (0, 0)
        a = 0
        for i in range(nch):
            k = base + (1 if i < rem else 0)
            if k:
                lst.append((a, k))
            a += k
        S.group_chunks.append(lst)
    S.CMAXG = int(max((k for lst in S.group_chunks for _, k in lst), default=1))

    per_core = []
    for ci in range(NC):
        ln, s, hi, order, lo_pad, hi_pad = core_data[ci]
        inv = np.empty(c.NPC, np.int64)
        inv[order] = np.arange(c.NPC)
        pos = inv[ln]
        A = np.zeros(S.SUMD * P, np.int16)
        for half, sel in (("lo", ~hi), ("hi", hi)):
            posh = pos[sel]
            rows = l1row[s[sel]]
            if half == "hi":
                rows = rows - c.ROW_HALF
            o2 = np.argsort(posh, kind="stable")
            posh_s = posh[o2]
            rows_s = rows[o2]
            starts = np.searchsorted(posh_s, np.arange(c.NPAD), side="left")
            j = np.arange(len(posh_s)) - starts[posh_s]
            gg = posh_s // P
            part = posh_s % P
            d = j + (S.L[gg] if half == "hi" else 0)
            assert (d < S.D[gg]).all()
            assert rows_s.max(initial=0) < 32768
            A[(S.Doff[gg] + d) * P + part] = rows_s.astype(np.int16)
        segs = []
        for g in range(c.G):
            n = int(S.D[g]) * P
            if n == 0:
                continue
            off = int(S.Doff[g]) * P
            segs.append(A[off : off + n].reshape(n // 16, 16).T)
        w = np.concatenate(segs, axis=1) if segs else np.zeros((16, 0), A.dtype)
        idxb = np.ascontiguousarray(np.tile(w, (8, 1)))

        mask = np.zeros((P, S.SUMD), np.float32)
        lo2 = lo_pad.reshape(c.G, P)
        hi2 = hi_pad.reshape(c.G, P)
        for g in range(c.G):
            d0 = int(S.Doff[g])
            L = int(S.L[g])
            H = int(S.H[g])
            if L:
                mask[:, d0 : d0 + L] = (
                    np.arange(L)[None, :] < lo2[g][:, None]
                ).astype(np.float32)
            if H:
                mask[:, d0 + L : d0 + L + H] = (
                    np.arange(H)[None, :] < hi2[g][:, None]
                ).astype(np.float32)
        xs = np.zeros((c.NPAD, c.IN), np.float32)
        xs[: c.NPC] = x[ci * c.NPC + order]
        per_core.append({"x": xs, "idxb": idxb, "maskb": mask})
    orders = [cd[3] for cd in core_data]
    return S, per_core, orders


def make_weights(cfg, W1, al1, ar1, b1, W2, al2, ar2, b2):
    c = cfg
    W1 = W1.astype(np.float64)
    A = np.zeros((c.IN, c.IN), np.float64)
    for h in range(c.H1):
        M = np.stack([al1[h], ar1[h]]).astype(np.float64)
        Qf, _ = np.linalg.qr(M.T, mode="complete")
        Ah = np.concatenate([M, Qf[:, 2:].T], axis=0)
        A[h * c.D1 : (h + 1) * c.D1, h * c.D1 : (h + 1) * c.D1] = Ah
    Ainv = np.linalg.inv(A)
    w1g = (W1 @ A.T).astype(np.float32)
    ainvt = np.ascontiguousarray(Ainv.T.astype(np.float32))
    w2g = np.zeros((c.IN, c.C2), np.float32)
    w2g[:, : c.O2] = W2
    w2g[:, c.O2] = W2.astype(np.float64) @ al2[0].astype(np.float64)
    w2g[:, c.O2 + 1] = W2.astype(np.float64) @ ar2[0].astype(np.float64)
    b1rep = np.ascontiguousarray(np.tile(b1[None, :], (P, 1)).astype(np.float32))
    b2rep = np.ascontiguousarray(np.tile(b2[None, :], (P, 1)).astype(np.float32))
    return {"w1g": w1g, "ainvt": ainvt, "w2g": w2g, "b1rep": b1rep, "b2rep": b2rep}


def _chunk_segments(S, g, a, k):
    L = int(S.L[g])
    segs = []
    lo_n = max(0, min(a + k, L) - a)
    if lo_n:
        segs.append(("lo", a, lo_n))
    hi_a = max(a, L)
    hi_n = a + k - hi_a
    if hi_n > 0:
        segs.append(("hi", hi_a, hi_n))
    return segs


def build_program(cfg, S, probe=False):
    c = cfg
    IN, C2, O2, H1, D1, G = c.IN, c.C2, c.O2, c.H1, c.D1, c.G
    AL = mybir.AluOpType
    AF = mybir.ActivationFunctionType
    AX = mybir.AxisListType
    CM = S.CMAXG

    ndev = 1 if probe else NC
    nc = bacc.Bacc("TRN2", target_bir_lowering=False, debug=False, num_devices=ndev, num_swdge_queues=4)
    xin = nc.dram_tensor("x", [c.NPAD, IN], F32, kind="ExternalInput")
    idxb = nc.dram_tensor("idxb", [P, 8 * max(S.SUMD, 1)], I16, kind="ExternalInput")
    maskb = nc.dram_tensor("maskb", [P, S.SUMD], F32, kind="ExternalInput")
    w1g_d = nc.dram_tensor("w1g", [IN, IN], F32, kind="ExternalInput")
    ainvt_d = nc.dram_tensor("ainvt", [IN, IN], F32, kind="ExternalInput")
    w2g_d = nc.dram_tensor("w2g", [IN, C2], F32, kind="ExternalInput")
    b1_d = nc.dram_tensor("b1rep", [P, IN], F32, kind="ExternalInput")
    b2_d = nc.dram_tensor("b2rep", [P, O2], F32, kind="ExternalInput")
    outp = nc.dram_tensor("out", [c.NPAD, O2], F32, kind="ExternalOutput")

    with tile.TileContext(nc) as tc:
        with (
            tc.tile_pool(name="dram", bufs=1, space="DRAM") as dram,
            tc.tile_pool(name="const", bufs=1) as const,
        ):
            bounce1 = dram.tile([c.NPAD, IN], F32)
            fs1 = dram.tile([NC * c.NPAD, IN], F32, addr_space="Shared")
            bounce2 = dram.tile([c.NPAD, C2], F32)
            fs2 = dram.tile([NC * c.NPAD, C2], F32, addr_space="Shared")

            ident = const.tile([P, P], F32)
            make_identity(nc, ident[:])
            w1g_a = const.tile([P, IN], F32)
            w1g_b = const.tile([P, IN], F32)
            nc.sync.dma_start(w1g_a[:], w1g_d[0:P, :])
            nc.sync.dma_start(w1g_b[:], w1g_d[P : 2 * P, :])
            ainvt_a = const.tile([P, IN], F32)
            ainvt_b = const.tile([P, IN], F32)
            nc.sync.dma_start(ainvt_a[:], ainvt_d[0:P, :])
            nc.sync.dma_start(ainvt_b[:], ainvt_d[P : 2 * P, :])
            w2g_a = const.tile([P, C2], F32)
            w2g_b = const.tile([P, C2], F32)
            nc.sync.dma_start(w2g_a[:], w2g_d[0:P, :])
            nc.sync.dma_start(w2g_b[:], w2g_d[P : 2 * P, :])
            b1_sb = const.tile([P, IN], F32)
            b2_sb = const.tile([P, O2], F32)
            nc.sync.dma_start(b1_sb[:], b1_d[:, :])
            nc.sync.dma_start(b2_sb[:], b2_d[:, :])
            mask_sb = const.tile([P, S.SUMD], F32)
            nc.sync.dma_start(mask_sb[:], maskb[:, :])
            er1tab = const.tile([P, G, H1], F32)
            er2tab = const.tile([P, G], F32)

            # ---- projection: g-table = x @ w1g ----
            with (
                tc.tile_pool(name="proj", bufs=3) as proj,
                tc.tile_pool(name="pp", bufs=2, space="PSUM") as pp,
            ):
                for t in range(G):
                    xt = proj.tile([P, IN], F32, tag="x")
                    nc.sync.dma_start(xt[:], xin[t * P : (t + 1) * P, :])
                    ps_t = pp.tile([P, IN], F32, tag="ps_t")
                    nc.tensor.transpose(ps_t[:, 0:P], xt[:, 0:P], ident[:])
                    nc.tensor.transpose(ps_t[:, P : 2 * P], xt[:, P : 2 * P], ident[:])
                    xT = proj.tile([P, IN], F32, tag="xT")
                    nc.scalar.copy(xT[:], ps_t[:])
                    ps_g = pp.tile([P, IN], F32, tag="ps_g")
                    nc.tensor.matmul(
                        out=ps_g[:], lhsT=xT[:, 0:P], rhs=w1g_a[:],
                        start=True, stop=False,
                    )
                    nc.tensor.matmul(
                        out=ps_g[:], lhsT=xT[:, P : 2 * P], rhs=w1g_b[:],
                        start=False, stop=True,
                    )
                    gsb = proj.tile([P, IN], F32, tag="gsb")
                    nc.vector.tensor_copy(gsb[:], ps_g[:])
                    nc.vector.tensor_copy(er1tab[:, t, :], gsb[:, 1 : IN : D1])
                    nc.sync.dma_start(bounce1[t * P : (t + 1) * P, :], gsb[:])

            if probe:
                nc.sync.dma_start(fs1[0 : c.NPAD, :], bounce1[:, :])
            else:
                nc.gpsimd.collective_compute(
                    "AllGather", mybir.AluOpType.bypass,
                    replica_groups=[list(range(NC))],
                    ins=[bounce1.opt()], outs=[fs1.opt()],
                )

            qctr = [0]
            # ---- layer-1 edge phase ----
            with (
                tc.tile_pool(name="edge", bufs=3) as ep,
                tc.tile_pool(name="msgp", bufs=1) as mp,
                tc.tile_pool(name="acc", bufs=2) as ac,
                tc.tile_pool(name="pp2", bufs=2, space="PSUM") as pp2,
            ):
                for g in range(G):
                    denom = ac.tile([P, H1], F32, tag="denom")
                    outg = ac.tile([P, IN], F32, tag="outg")
                    first = True
                    for a, k in S.group_chunks[g]:
                        moff = int(S.Doff[g]) + a
                        idxs = ep.tile([P, 8 * CM], I16, tag="idx")
                        nc.sync.dma_start(
                            idxs[:, 0 : 8 * k], idxb[:, 8 * moff : 8 * (moff + k)]
                        )
                        gb = ep.tile([P, CM, IN], F32, tag="gb")
                        for half, s0, n in _chunk_segments(S, g, a, k):
                            src_ap = (
                                fs1[0 : c.ROW_HALF, :]
                                if half == "lo"
                                else fs1[c.ROW_HALF : 2 * c.ROW_HALF, :]
                            )
                            o = s0 - a
                            nc.gpsimd.dma_gather(
                                gb[:, o : o + n, :], src_ap,
                                idxs[:, 8 * o : 8 * (o + n)],
                                P * n, P * n, IN, single_packet=False,
                                queue_num=qctr[0] % 4,
                            )
                            qctr[0] += 1
                        e1 = ep.tile([P, CM, H1], F32, tag="e1")
                        nc.vector.tensor_tensor(
                            out=e1[:, 0:k, :],
                            in0=gb[:, 0:k, 0 : IN : D1],
                            in1=er1tab[:, g, None, :].to_broadcast([P, k, H1]),
                            op=AL.add,
                        )
                        e2 = ep.tile([P, CM, H1], F32, tag="e2")
                        nc.vector.scalar_tensor_tensor(
                            out=e2[:, 0:k, :], in0=e1[:, 0:k, :], scalar=NEG,
                            in1=e1[:, 0:k, :], op0=AL.mult, op1=AL.max,
                        )
                        e3 = ep.tile([P, H1, CM], F32, tag="e3")
                        nc.scalar.activation(
                            e3[:, :, 0:k].rearrange("p h j -> p j h"),
                            e2[:, 0:k, :], AF.Exp,
                        )
                        exm = ep.tile([P, H1, CM], F32, tag="exm")
                        nc.vector.tensor_tensor(
                            out=exm[:, :, 0:k],
                            in0=e3[:, :, 0:k],
                            in1=mask_sb[:, None, moff : moff + k].to_broadcast(
                                [P, H1, k]
                            ),
                            op=AL.mult,
                        )
                        if first:
                            nc.vector.tensor_reduce(
                                out=denom[:, :], in_=exm[:, :, 0:k], axis=AX.X, op=AL.add
                            )
                        else:
                            dtmp = ep.tile([P, H1], F32, tag="dtmp")
                            nc.vector.tensor_reduce(
                                out=dtmp[:, :], in_=exm[:, :, 0:k], axis=AX.X, op=AL.add
                            )
                            nc.vector.tensor_tensor(
                                out=denom[:, :], in0=denom[:, :], in1=dtmp[:, :], op=AL.add
                            )
                        msg = mp.tile([P, IN, CM], F32, tag="msg")
                        nc.vector.tensor_tensor(
                            out=msg[:, :, 0:k].rearrange("p (h d) j -> p h d j", h=H1),
                            in0=gb[:, 0:k, :].rearrange("p j (h d) -> p h d j", h=H1),
                            in1=exm[:, :, None, 0:k].to_broadcast([P, H1, D1, k]),
                            op=AL.mult,
                        )
                        if first:
                            nc.vector.tensor_reduce(
                                out=outg[:, :], in_=msg[:, :, 0:k], axis=AX.X, op=AL.add
                            )
                        else:
                            otmp = ep.tile([P, IN], F32, tag="otmp")
                            nc.vector.tensor_reduce(
                                out=otmp[:, :], in_=msg[:, :, 0:k], axis=AX.X, op=AL.add
                            )
                            nc.vector.tensor_tensor(
                                out=outg[:, :], in0=outg[:, :], in1=otmp[:, :], op=AL.add
                            )
                        first = False
                    if first:
                        nc.vector.memset(denom[:, :], 1.0)
                        nc.vector.memset(outg[:, :], 0.0)
                    rden = ac.tile([P, H1], F32, tag="rden")
                    nc.vector.reciprocal(rden[:, :], denom[:, :])
                    outn = ac.tile([P, IN], F32, tag="outn")
                    nc.vector.tensor_tensor(
                        out=outn[:, :].rearrange("p (h d) -> p h d", h=H1),
                        in0=outg[:, :].rearrange("p (h d) -> p h d", h=H1),
                        in1=rden[:, :, None].to_broadcast([P, H1, D1]),
                        op=AL.mult,
                    )
                    ps_a = pp2.tile([P, IN], F32, tag="ps_a")
                    nc.tensor.transpose(ps_a[:, 0:P], outn[:, 0:P], ident[:])
                    nc.tensor.transpose(ps_a[:, P : 2 * P], outn[:, P : 2 * P], ident[:])
                    gT = ac.tile([P, IN], F32, tag="gT")
                    nc.scalar.copy(gT[:], ps_a[:])
                    ps_f = pp2.tile([P, IN], F32, tag="ps_f")
                    nc.tensor.matmul(
                        out=ps_f[:], lhsT=gT[:, 0:P], rhs=ainvt_a[:],
                        start=True, stop=False,
                    )
                    nc.tensor.matmul(
                        out=ps_f[:], lhsT=gT[:, P : 2 * P], rhs=ainvt_b[:],
                        start=False, stop=True,
                    )
                    hq = ac.tile([P, IN], F32, tag="hq")
                    nc.vector.tensor_add(hq[:], ps_f[:], b1_sb[:])
                    hr = ac.tile([P, IN], F32, tag="hr")
                    nc.scalar.activation(hr[:], hq[:], AF.Relu)
                    ps_b = pp2.tile([P, IN], F32, tag="ps_a")
                    nc.tensor.transpose(ps_b[:, 0:P], hr[:, 0:P], ident[:])
                    nc.tensor.transpose(ps_b[:, P : 2 * P], hr[:, P : 2 * P], ident[:])
                    hT = ac.tile([P, IN], F32, tag="hT")
                    nc.scalar.copy(hT[:], ps_b[:])
                    ps_2 = pp2.tile([P, C2], F32, tag="ps_2")
                    nc.tensor.matmul(
                        out=ps_2[:], lhsT=hT[:, 0:P], rhs=w2g_a[:],
                        start=True, stop=False,
                    )
                    nc.tensor.matmul(
                        out=ps_2[:], lhsT=hT[:, P : 2 * P], rhs=w2g_b[:],
                        start=False, stop=True,
                    )
                    f2 = ac.tile([P, C2], F32, tag="f2")
                    nc.vector.tensor_copy(f2[:], ps_2[:])
                    nc.vector.tensor_copy(er2tab[:, g : g + 1], f2[:, O2 + 1 : O2 + 2])
                    nc.sync.dma_start(bounce2[g * P : (g + 1) * P, :], f2[:])

            if probe:
                nc.sync.dma_start(fs2[0 : c.NPAD, :], bounce2[:, :])
            else:
                nc.gpsimd.collective_compute(
                    "AllGather", mybir.AluOpType.bypass,
                    replica_groups=[list(range(NC))],
                    ins=[bounce2.opt()], outs=[fs2.opt()],
                )

            # ---- layer-2 edge phase ----
            with (
                tc.tile_pool(name="edge2", bufs=3) as ep,
                tc.tile_pool(name="msgp2", bufs=1) as mp,
                tc.tile_pool(name="acc2", bufs=2) as ac,
            ):
                for g in range(G):
                    denom = ac.tile([P, 1], F32, tag="denom")
                    out2 = ac.tile([P, O2], F32, tag="out2")
                    first = True
                    for a, k in S.group_chunks[g]:
                        moff = int(S.Doff[g]) + a
                        idxs = ep.tile([P, 8 * CM], I16, tag="idx")
                        nc.sync.dma_start(
                            idxs[:, 0 : 8 * k], idxb[:, 8 * moff : 8 * (moff + k)]
                        )
                        gb = ep.tile([P, CM, C2], F32, tag="gb")
                        for half, s0, n in _chunk_segments(S, g, a, k):
                            src_ap = (
                                fs2[0 : c.ROW_HALF, :]
                                if half == "lo"
                                else fs2[c.ROW_HALF : 2 * c.ROW_HALF, :]
                            )
                            o = s0 - a
                            nc.gpsimd.dma_gather(
                                gb[:, o : o + n, :], src_ap,
                                idxs[:, 8 * o : 8 * (o + n)],
                                P * n, P * n, C2, single_packet=False,
                                queue_num=qctr[0] % 4,
                            )
                            qctr[0] += 1
                        e1 = ep.tile([P, CM], F32, tag="e1")
                        nc.vector.tensor_tensor(
                            out=e1[:, 0:k],
                            in0=gb[:, 0:k, O2],
                            in1=er2tab[:, g : g + 1].to_broadcast([P, k]),
                            op=AL.add,
                        )
                        e2 = ep.tile([P, CM], F32, tag="e2")
                        nc.vector.scalar_tensor_tensor(
                            out=e2[:, 0:k], in0=e1[:, 0:k], scalar=NEG,
                            in1=e1[:, 0:k], op0=AL.mult, op1=AL.max,
                        )
                        e3 = ep.tile([P, CM], F32, tag="e3")
                        nc.scalar.activation(e3[:, 0:k], e2[:, 0:k], AF.Exp)
                        exm = ep.tile([P, CM], F32, tag="exm")
                        nc.vector.tensor_tensor(
                            out=exm[:, 0:k], in0=e3[:, 0:k],
                            in1=mask_sb[:, moff : moff + k], op=AL.mult,
                        )
                        if first:
                            nc.vector.tensor_reduce(
                                out=denom[:, :], in_=exm[:, 0:k], axis=AX.X, op=AL.add
                            )
                        else:
                            dtmp = ep.tile([P, 1], F32, tag="dtmp")
                            nc.vector.tensor_reduce(
                                out=dtmp[:, :], in_=exm[:, 0:k], axis=AX.X, op=AL.add
                            )
                            nc.vector.tensor_tensor(
                                out=denom[:, :], in0=denom[:, :], in1=dtmp[:, :], op=AL.add
                            )
                        msg = mp.tile([P, O2, CM], F32, tag="msg")
                        nc.vector.tensor_tensor(
                            out=msg[:, :, 0:k],
                            in0=gb[:, 0:k, 0:O2].rearrange("p j f -> p f j"),
                            in1=exm[:, None, 0:k].to_broadcast([P, O2, k]),
                            op=AL.mult,
                        )
                        if first:
                            nc.vector.tensor_reduce(
                                out=out2[:, :], in_=msg[:, :, 0:k], axis=AX.X, op=AL.add
                            )
                        else:
                            otmp = ep.tile([P, O2], F32, tag="otmp")
                            nc.vector.tensor_reduce(
                                out=otmp[:, :], in_=msg[:, :, 0:k], axis=AX.X, op=AL.add
                            )
                            nc.vector.tensor_tensor(
                                out=out2[:, :], in0=out2[:, :], in1=otmp[:, :], op=AL.add
                            )
                        first = False
                    if first:
                        nc.vector.memset(denom[:, :], 1.0)
                        nc.vector.memset(out2[:, :], 0.0)
                    rden = ac.tile([P, 1], F32, tag="rden")
                    nc.vector.reciprocal(rden[:, :], denom[:, :])
                    on = ac.tile([P, O2], F32, tag="on")
                    nc.vector.tensor_scalar_mul(on[:], out2[:], rden[:, 0:1])
                    ob = ac.tile([P, O2], F32, tag="ob")
                    nc.vector.tensor_add(ob[:], on[:], b2_sb[:])
                    nc.sync.dma_start(outp[g * P : (g + 1) * P, :], ob[:])
    nc.compile()
    return nc



# ---------------------------------------------------------------- runner with
# persistent executable (avoids re-jit on repeated kernel() calls)
import os as _os
_os.environ.setdefault("JAX_COMPILATION_CACHE_DIR", "/tmp/jax_neff_cache")

N_NODES = 50000
N_EDGES = 800000

_CACHE = {}


class _Runner:
    def __init__(self, cfg, S):
        import jax
        from jax.sharding import Mesh, PartitionSpec
        from jax.experimental.shard_map import shard_map
        import concourse.bass2jax as b2j

        self.cfg = cfg
        nc = build_program(cfg, S)
        b2j.install_neuronx_cc_hook()
        partition_name = (
            nc.partition_id_tensor.name if nc.partition_id_tensor else None
        )
        in_names, out_names, out_avals, zero_outs = [], [], [], []
        for alloc in nc.m.functions[0].allocations:
            if not isinstance(alloc, mybir.MemoryLocationSet):
                continue
            name = alloc.memorylocations[0].name
            if alloc.kind == "ExternalInput":
                if name != partition_name:
                    in_names.append(name)
            elif alloc.kind == "ExternalOutput":
                out_names.append(name)
                shape = tuple(alloc.tensor_shape)
                dtype = mybir.dt.np(alloc.dtype)
                out_avals.append(jax.core.ShapedArray(shape, dtype))
                zero_outs.append(np.zeros(shape, dtype))
        self.n_params = len(in_names)
        self.param_names = list(in_names)
        self.out_names = out_names
        self.zero_outs = zero_outs
        all_in = in_names + out_names
        if partition_name is not None:
            all_in.append(partition_name)

        def _body(*args):
            operands = list(args)
            if partition_name is not None:
                operands.append(b2j.partition_id_tensor())
            outs = b2j._bass_exec_p.bind(
                *operands,
                out_avals=tuple(out_avals),
                in_names=tuple(all_in),
                out_names=tuple(out_names),
                lowering_input_output_aliases=(),
                sim_require_finite=True,
                sim_require_nnan=True,
                nc=nc,
            )
            return tuple(outs)

        devices = jax.devices()[:NC]
        assert len(devices) == NC
        self.mesh = Mesh(np.asarray(devices), ("core",))
        n_io = self.n_params + len(out_names)
        self.fn = jax.jit(
            shard_map(
                _body,
                mesh=self.mesh,
                in_specs=(PartitionSpec("core"),) * n_io,
                out_specs=(PartitionSpec("core"),) * len(out_names),
                check_rep=False,
            ),
            keep_unused=True,
        )
        self.jax = jax
        self.PartitionSpec = PartitionSpec

    def put_inputs(self, in_maps):
        jax = self.jax
        from jax.sharding import NamedSharding

        sh = NamedSharding(self.mesh, self.PartitionSpec("core"))
        args = []
        for name in self.param_names:
            g = np.concatenate([np.asarray(m[name]) for m in in_maps], axis=0)
            args.append(jax.device_put(g, sh))
        for z in self.zero_outs:
            g = np.zeros((NC * z.shape[0], *z.shape[1:]), z.dtype)
            args.append(jax.device_put(g, sh))
        return args

    def __call__(self, args):
        outs = self.fn(*args)
        return [np.asarray(o) for o in outs]


def _get_runner(cfg, S, key):
    r = _CACHE.get(key)
    if r is None:
        r = _Runner(cfg, S)
        _CACHE[key] = r
    return r


def _prepare(inputs):
    cfg = Cfg(N_NODES, N_EDGES)
    x = np.ascontiguousarray(np.asarray(inputs["x"], np.float32))
    src = np.asarray(inputs["src"])
    dst = np.asarray(inputs["dst"])
    S, per_core, orders = preprocess(cfg, x, src, dst)
    shared = make_weights(
        cfg,
        *(np.asarray(inputs[k], np.float32)
          for k in ("W1", "al1", "ar1", "b1", "W2", "al2", "ar2", "b2")),
    )
    in_maps = [dict(shared, **pc) for pc in per_core]
    import hashlib

    key = hashlib.sha1(src.tobytes() + dst.tobytes()).hexdigest()
    return cfg, S, in_maps, orders, key


def kernel(**inputs) -> np.ndarray:
    cfg, S, in_maps, orders, key = _prepare(inputs)
    runner = _get_runner(cfg, S, key)
    args = runner.put_inputs(in_maps)
    outs = runner(args)
    full = outs[runner.out_names.index("out")].reshape(NC, cfg.NPAD, cfg.O2)
    out = np.empty((cfg.N, cfg.O2), np.float32)
    for ci in range(NC):
        out[ci * cfg.NPC + orders[ci]] = full[ci, : cfg.NPC]
    return out


# revision 6
# speedup vs baseline: 1.0638x; 1.0638x over previous
"""GAT 2-layer GNN on 8 Trainium2 NeuronCores (Bass/Tile). Dev module (v2).

Sharding: nodes partitioned contiguously across 8 cores by dst ownership.
Per core, owned nodes are degree-sorted ((max,min) of lo/hi in-degrees) and
processed in 128-node groups: node-on-partition layout, incoming-edge slots
along the free axis. Slots [0,L) gather from the low table half, [L,L+H)
from the high half (int16 dma_gather index limit). Layer-1 per-head feature
blocks live in a rotated basis whose first two coordinates are el/er, so one
1024-byte gather row per edge carries features + logits; the aggregate is
rotated back by a block-diagonal matmul on the TensorEngine. The layer-1 ->
layer-2 halo exchange is a single AllGather collective.
"""
import sys

sys.path.insert(0, "/opt/trn_rl_repo")
import numpy as np

import concourse.bass as bass
import concourse.bacc as bacc
import concourse.mybir as mybir
import concourse.tile as tile
from concourse.bass_utils import run_bass_kernel_spmd
from concourse.masks import make_identity

F32 = mybir.dt.float32
I16 = mybir.dt.int16

NC = 8
P = 128
NEG = 0.2
CMAX = 40  # max slots per compute chunk


class Cfg:
    def __init__(self, N, E, IN=256, H1=4, D1=64, O2=47):
        assert N % NC == 0
        self.N, self.E, self.IN, self.H1, self.D1, self.O2 = N, E, IN, H1, D1, O2
        assert H1 * D1 == IN
        self.NPC = N // NC
        self.G = -(-self.NPC // P)
        self.NPAD = self.G * P
        self.HALF_NODE = N // 2
        self.ROW_HALF = (NC // 2) * self.NPAD
        self.C2 = 64


class Sched:
    pass


def preprocess(cfg, x, src, dst):
    """Per-core shards + shared schedule. dst must be sorted. The index blob
    is a single [128, 8*SUMD] int16 array whose column offset for slot d of
    group g is 8*(Doff[g]+d) — identical to the mask column layout."""
    c = cfg
    e0 = np.searchsorted(dst, np.arange(NC + 1) * c.NPC).astype(np.int64)

    l1row = np.empty(c.N, np.int64)
    core_data = []
    Ls = np.zeros((NC, c.G), np.int64)
    Hs = np.zeros((NC, c.G), np.int64)
    for ci in range(NC):
        sl = slice(e0[ci], e0[ci + 1])
        ln = (dst[sl] - ci * c.NPC).astype(np.int64)
        s = src[sl].astype(np.int64)
        hi = s >= c.HALF_NODE
        lo_deg = np.bincount(ln[~hi], minlength=c.NPC)
        hi_deg = np.bincount(ln[hi], minlength=c.NPC)
        order = np.lexsort(
            (np.minimum(lo_deg, hi_deg), np.maximum(lo_deg, hi_deg))
        )
        l1row[ci * c.NPC + order] = ci * c.NPAD + np.arange(c.NPC)
        lo_pad = np.zeros(c.NPAD, np.int64)
        hi_pad = np.zeros(c.NPAD, np.int64)
        lo_pad[: c.NPC] = lo_deg[order]
        hi_pad[: c.NPC] = hi_deg[order]
        Ls[ci] = lo_pad.reshape(c.G, P).max(1)
        Hs[ci] = hi_pad.reshape(c.G, P).max(1)
        core_data.append((ln, s, hi, order, lo_pad, hi_pad))

    S = Sched()
    S.L = Ls.max(0)
    S.H = Hs.max(0)
    S.D = S.L + S.H
    S.Doff = np.concatenate([[0], np.cumsum(S.D)])
    S.SUMD = int(S.D.sum())
    S.group_chunks = []
    for g in range(c.G):
        D = int(S.D[g])
        lst = []
        nch = max(1, -(-D // CMAX))
        base, rem = divmod(D, nch) if nch else (0, 0)
        a = 0
        for i in range(nch):
            k = base + (1 if i < rem else 0)
            if k:
                lst.append((a, k))
            a += k
        S.group_chunks.append(lst)
    S.CMAXG = int(max((k for lst in S.group_chunks for _, k in lst), default=1))

    per_core = []
    for ci in range(NC):
        ln, s, hi, order, lo_pad, hi_pad = core_data[ci]
        inv = np.empty(c.NPC, np.int64)
        inv[order] = np.arange(c.NPC)
        pos = inv[ln]
        A = np.zeros(S.SUMD * P, np.int16)
        for half, sel in (("lo", ~hi), ("hi", hi)):
            posh = pos[sel]
            rows = l1row[s[sel]]
            if half == "hi":
                rows = rows - c.ROW_HALF
            o2 = np.argsort(posh, kind="stable")
            posh_s = posh[o2]
            rows_s = rows[o2]
            starts = np.searchsorted(posh_s, np.arange(c.NPAD), side="left")
            j = np.arange(len(posh_s)) - starts[posh_s]
            gg = posh_s // P
            part = posh_s % P
            d = j + (S.L[gg] if half == "hi" else 0)
            assert (d < S.D[gg]).all()
            assert rows_s.max(initial=0) < 32768
            A[(S.Doff[gg] + d) * P + part] = rows_s.astype(np.int16)
        segs = []
        for g in range(c.G):
            n = int(S.D[g]) * P
            if n == 0:
                continue
            off = int(S.Doff[g]) * P
            segs.append(A[off : off + n].reshape(n // 16, 16).T)
        w = np.concatenate(segs, axis=1) if segs else np.zeros((16, 0), A.dtype)
        idxb = np.ascontiguousarray(np.tile(w, (8, 1)))

        mask = np.zeros((P, S.SUMD), np.float32)
        lo2 = lo_pad.reshape(c.G, P)
        hi2 = hi_pad.reshape(c.G, P)
        for g in range(c.G):
            d0 = int(S.Doff[g])
            L = int(S.L[g])
            H = int(S.H[g])
            if L:
                mask[:, d0 : d0 + L] = (
                    np.arange(L)[None, :] < lo2[g][:, None]
                ).astype(np.float32)
            if H:
                mask[:, d0 + L : d0 + L + H] = (
                    np.arange(H)[None, :] < hi2[g][:, None]
                ).astype(np.float32)
        xs = np.zeros((c.NPAD, c.IN), np.float32)
        xs[: c.NPC] = x[ci * c.NPC + order]
        per_core.append({"x": xs, "idxb": idxb, "maskb": mask})
    orders = [cd[3] for cd in core_data]
    return S, per_core, orders


def make_weights(cfg, W1, al1, ar1, b1, W2, al2, ar2, b2):
    c = cfg
    W1 = W1.astype(np.float64)
    A = np.zeros((c.IN, c.IN), np.float64)
    for h in range(c.H1):
        M = np.stack([al1[h], ar1[h]]).astype(np.float64)
        Qf, _ = np.linalg.qr(M.T, mode="complete")
        Ah = np.concatenate([M, Qf[:, 2:].T], axis=0)
        A[h * c.D1 : (h + 1) * c.D1, h * c.D1 : (h + 1) * c.D1] = Ah
    Ainv = np.linalg.inv(A)
    w1g = (W1 @ A.T).astype(np.float32)
    ainvt = np.ascontiguousarray(Ainv.T.astype(np.float32))
    w2g = np.zeros((c.IN, c.C2), np.float32)
    w2g[:, : c.O2] = W2
    w2g[:, c.O2] = W2.astype(np.float64) @ al2[0].astype(np.float64)
    w2g[:, c.O2 + 1] = W2.astype(np.float64) @ ar2[0].astype(np.float64)
    b1rep = np.ascontiguousarray(np.tile(b1[None, :], (P, 1)).astype(np.float32))
    b2rep = np.ascontiguousarray(np.tile(b2[None, :], (P, 1)).astype(np.float32))
    return {"w1g": w1g, "ainvt": ainvt, "w2g": w2g, "b1rep": b1rep, "b2rep": b2rep}


def _chunk_segments(S, g, a, k):
    L = int(S.L[g])
    segs = []
    lo_n = max(0, min(a + k, L) - a)
    if lo_n:
        segs.append(("lo", a, lo_n))
    hi_a = max(a, L)
    hi_n = a + k - hi_a
    if hi_n > 0:
        segs.append(("hi", hi_a, hi_n))
    return segs


def build_program(cfg, S, probe=False):
    c = cfg
    IN, C2, O2, H1, D1, G = c.IN, c.C2, c.O2, c.H1, c.D1, c.G
    AL = mybir.AluOpType
    AF = mybir.ActivationFunctionType
    AX = mybir.AxisListType
    CM = S.CMAXG

    ndev = 1 if probe else NC
    nc = bacc.Bacc("TRN2", target_bir_lowering=False, debug=False, num_devices=ndev, num_swdge_queues=4)
    xin = nc.dram_tensor("x", [c.NPAD, IN], F32, kind="ExternalInput")
    idxb = nc.dram_tensor("idxb", [P, 8 * max(S.SUMD, 1)], I16, kind="ExternalInput")
    maskb = nc.dram_tensor("maskb", [P, S.SUMD], F32, kind="ExternalInput")
    w1g_d = nc.dram_tensor("w1g", [IN, IN], F32, kind="ExternalInput")
    ainvt_d = nc.dram_tensor("ainvt", [IN, IN], F32, kind="ExternalInput")
    w2g_d = nc.dram_tensor("w2g", [IN, C2], F32, kind="ExternalInput")
    b1_d = nc.dram_tensor("b1rep", [P, IN], F32, kind="ExternalInput")
    b2_d = nc.dram_tensor("b2rep", [P, O2], F32, kind="ExternalInput")
    outp = nc.dram_tensor("out", [c.NPAD, O2], F32, kind="ExternalOutput")

    with tile.TileContext(nc) as tc:
        with (
            tc.tile_pool(name="dram", bufs=1, space="DRAM") as dram,
            tc.tile_pool(name="const", bufs=1) as const,
        ):
            bounce1 = dram.tile([c.NPAD, IN], F32)
            fs1 = dram.tile([NC * c.NPAD, IN], F32, addr_space="Shared")
            bounce2 = dram.tile([c.NPAD, C2], F32)
            fs2 = dram.tile([NC * c.NPAD, C2], F32, addr_space="Shared")

            ident = const.tile([P, P], F32)
            make_identity(nc, ident[:])
            w1g_a = const.tile([P, IN], F32)
            w1g_b = const.tile([P, IN], F32)
            nc.sync.dma_start(w1g_a[:], w1g_d[0:P, :])
            nc.sync.dma_start(w1g_b[:], w1g_d[P : 2 * P, :])
            ainvt_a = const.tile([P, IN], F32)
            ainvt_b = const.tile([P, IN], F32)
            nc.sync.dma_start(ainvt_a[:], ainvt_d[0:P, :])
            nc.sync.dma_start(ainvt_b[:], ainvt_d[P : 2 * P, :])
            w2g_a = const.tile([P, C2], F32)
            w2g_b = const.tile([P, C2], F32)
            nc.sync.dma_start(w2g_a[:], w2g_d[0:P, :])
            nc.sync.dma_start(w2g_b[:], w2g_d[P : 2 * P, :])
            b1_sb = const.tile([P, IN], F32)
            b2_sb = const.tile([P, O2], F32)
            nc.sync.dma_start(b1_sb[:], b1_d[:, :])
            nc.sync.dma_start(b2_sb[:], b2_d[:, :])
            mask_sb = const.tile([P, S.SUMD], F32)
            nc.sync.dma_start(mask_sb[:], maskb[:, :])
            er1tab = const.tile([P, G, H1], F32)
            er2tab = const.tile([P, G], F32)

            # ---- projection: g-table = x @ w1g ----
            with (
                tc.tile_pool(name="proj", bufs=3) as proj,
                tc.tile_pool(name="pp", bufs=2, space="PSUM") as pp,
            ):
                for t in range(G):
                    xt = proj.tile([P, IN], F32, tag="x")
                    nc.sync.dma_start(xt[:], xin[t * P : (t + 1) * P, :])
                    ps_t = pp.tile([P, IN], F32, tag="ps_t")
                    nc.tensor.transpose(ps_t[:, 0:P], xt[:, 0:P], ident[:])
                    nc.tensor.transpose(ps_t[:, P : 2 * P], xt[:, P : 2 * P], ident[:])
                    xT = proj.tile([P, IN], F32, tag="xT")
                    nc.scalar.copy(xT[:], ps_t[:])
                    ps_g = pp.tile([P, IN], F32, tag="ps_g")
                    nc.tensor.matmul(
                        out=ps_g[:], lhsT=xT[:, 0:P], rhs=w1g_a[:],
                        start=True, stop=False,
                    )
                    nc.tensor.matmul(
                        out=ps_g[:], lhsT=xT[:, P : 2 * P], rhs=w1g_b[:],
                        start=False, stop=True,
                    )
                    gsb = proj.tile([P, IN], F32, tag="gsb")
                    nc.vector.tensor_copy(gsb[:], ps_g[:])
                    nc.vector.tensor_copy(er1tab[:, t, :], gsb[:, 1 : IN : D1])
                    nc.sync.dma_start(bounce1[t * P : (t + 1) * P, :], gsb[:])

            if probe:
                nc.sync.dma_start(fs1[0 : c.NPAD, :], bounce1[:, :])
            else:
                nc.gpsimd.collective_compute(
                    "AllGather", mybir.AluOpType.bypass,
                    replica_groups=[list(range(NC))],
                    ins=[bounce1.opt()], outs=[fs1.opt()],
                )

            qctr = [0]
            # ---- layer-1 edge phase ----
            with (
                tc.tile_pool(name="edge", bufs=3) as ep,
                tc.tile_pool(name="msgp", bufs=1) as mp,
                tc.tile_pool(name="acc", bufs=2) as ac,
                tc.tile_pool(name="pp2", bufs=2, space="PSUM") as pp2,
            ):
                for g in range(G):
                    denom = ac.tile([P, H1], F32, tag="denom")
                    outg = ac.tile([P, IN], F32, tag="outg")
                    first = True
                    for a, k in S.group_chunks[g]:
                        moff = int(S.Doff[g]) + a
                        idxs = ep.tile([P, 8 * CM], I16, tag="idx")
                        nc.sync.dma_start(
                            idxs[:, 0 : 8 * k], idxb[:, 8 * moff : 8 * (moff + k)]
                        )
                        gb = ep.tile([P, CM, IN], F32, tag="gb")
                        for half, s0, n in _chunk_segments(S, g, a, k):
                            src_ap = (
                                fs1[0 : c.ROW_HALF, :]
                                if half == "lo"
                                else fs1[c.ROW_HALF : 2 * c.ROW_HALF, :]
                            )
                            o = s0 - a
                            nc.gpsimd.dma_gather(
                                gb[:, o : o + n, :], src_ap,
                                idxs[:, 8 * o : 8 * (o + n)],
                                P * n, P * n, IN, single_packet=False,
                                queue_num=qctr[0] % 4,
                            )
                            qctr[0] += 1
                        e1 = ep.tile([P, CM, H1], F32, tag="e1")
                        nc.vector.tensor_tensor(
                            out=e1[:, 0:k, :],
                            in0=gb[:, 0:k, 0 : IN : D1],
                            in1=er1tab[:, g, None, :].to_broadcast([P, k, H1]),
                            op=AL.add,
                        )
                        e2 = ep.tile([P, CM, H1], F32, tag="e2")
                        nc.vector.scalar_tensor_tensor(
                            out=e2[:, 0:k, :], in0=e1[:, 0:k, :], scalar=NEG,
                            in1=e1[:, 0:k, :], op0=AL.mult, op1=AL.max,
                        )
                        e3 = ep.tile([P, H1, CM], F32, tag="e3")
                        nc.scalar.activation(
                            e3[:, :, 0:k].rearrange("p h j -> p j h"),
                            e2[:, 0:k, :], AF.Exp,
                        )
                        exm = ep.tile([P, H1, CM], F32, tag="exm")
                        nc.vector.tensor_tensor(
                            out=exm[:, :, 0:k],
                            in0=e3[:, :, 0:k],
                            in1=mask_sb[:, None, moff : moff + k].to_broadcast(
                                [P, H1, k]
                            ),
                            op=AL.mult,
                        )
                        if first:
                            nc.vector.tensor_reduce(
                                out=denom[:, :], in_=exm[:, :, 0:k], axis=AX.X, op=AL.add
                            )
                        else:
                            dtmp = ep.tile([P, H1], F32, tag="dtmp")
                            nc.vector.tensor_reduce(
                                out=dtmp[:, :], in_=exm[:, :, 0:k], axis=AX.X, op=AL.add
                            )
                            nc.vector.tensor_tensor(
                                out=denom[:, :], in0=denom[:, :], in1=dtmp[:, :], op=AL.add
                            )
                        msg = mp.tile([P, IN, CM], F32, tag="msg")
                        nc.vector.tensor_tensor(
                            out=msg[:, :, 0:k].rearrange("p (h d) j -> p h d j", h=H1),
                            in0=gb[:, 0:k, :].rearrange("p j (h d) -> p h d j", h=H1),
                            in1=exm[:, :, None, 0:k].to_broadcast([P, H1, D1, k]),
                            op=AL.mult,
                        )
                        if first:
                            nc.vector.tensor_reduce(
                                out=outg[:, :], in_=msg[:, :, 0:k], axis=AX.X, op=AL.add
                            )
                        else:
                            otmp = ep.tile([P, IN], F32, tag="otmp")
                            nc.vector.tensor_reduce(
                                out=otmp[:, :], in_=msg[:, :, 0:k], axis=AX.X, op=AL.add
                            )
                            nc.vector.tensor_tensor(
                                out=outg[:, :], in0=outg[:, :], in1=otmp[:, :], op=AL.add
                            )
                        first = False
                    if first:
                        nc.vector.memset(denom[:, :], 1.0)
                        nc.vector.memset(outg[:, :], 0.0)
                    rden = ac.tile([P, H1], F32, tag="rden")
                    nc.vector.reciprocal(rden[:, :], denom[:, :])
                    outn = ac.tile([P, IN], F32, tag="outn")
                    nc.vector.tensor_tensor(
                        out=outn[:, :].rearrange("p (h d) -> p h d", h=H1),
                        in0=outg[:, :].rearrange("p (h d) -> p h d", h=H1),
                        in1=rden[:, :, None].to_broadcast([P, H1, D1]),
                        op=AL.mult,
                    )
                    ps_a = pp2.tile([P, IN], F32, tag="ps_a")
                    nc.tensor.transpose(ps_a[:, 0:P], outn[:, 0:P], ident[:])
                    nc.tensor.transpose(ps_a[:, P : 2 * P], outn[:, P : 2 * P], ident[:])
                    gT = ac.tile([P, IN], F32, tag="gT")
                    nc.scalar.copy(gT[:], ps_a[:])
                    ps_f = pp2.tile([P, IN], F32, tag="ps_f")
                    nc.tensor.matmul(
                        out=ps_f[:], lhsT=gT[:, 0:P], rhs=ainvt_a[:],
                        start=True, stop=False,
                    )
                    nc.tensor.matmul(
                        out=ps_f[:], lhsT=gT[:, P : 2 * P], rhs=ainvt_b[:],
                        start=False, stop=True,
                    )
                    hq = ac.tile([P, IN], F32, tag="hq")
                    nc.vector.tensor_add(hq[:], ps_f[:], b1_sb[:])
                    hr = ac.tile([P, IN], F32, tag="hr")
                    nc.scalar.activation(hr[:], hq[:], AF.Relu)
                    ps_b = pp2.tile([P, IN], F32, tag="ps_a")
                    nc.tensor.transpose(ps_b[:, 0:P], hr[:, 0:P], ident[:])
                    nc.tensor.transpose(ps_b[:, P : 2 * P], hr[:, P : 2 * P], ident[:])
                    hT = ac.tile([P, IN], F32, tag="hT")
                    nc.scalar.copy(hT[:], ps_b[:])
                    ps_2 = pp2.tile([P, C2], F32, tag="ps_2")
                    nc.tensor.matmul(
                        out=ps_2[:], lhsT=hT[:, 0:P], rhs=w2g_a[:],
                        start=True, stop=False,
                    )
                    nc.tensor.matmul(
                        out=ps_2[:], lhsT=hT[:, P : 2 * P], rhs=w2g_b[:],
                        start=False, stop=True,
                    )
                    f2 = ac.tile([P, C2], F32, tag="f2")
                    nc.vector.tensor_copy(f2[:], ps_2[:])
                    nc.vector.tensor_copy(er2tab[:, g : g + 1], f2[:, O2 + 1 : O2 + 2])
                    nc.sync.dma_start(bounce2[g * P : (g + 1) * P, :], f2[:])

            if probe:
                nc.sync.dma_start(fs2[0 : c.NPAD, :], bounce2[:, :])
            else:
                nc.gpsimd.collective_compute(
                    "AllGather", mybir.AluOpType.bypass,
                    replica_groups=[list(range(NC))],
                    ins=[bounce2.opt()], outs=[fs2.opt()],
                )

            # ---- layer-2 edge phase ----
            with (
                tc.tile_pool(name="edge2", bufs=3) as ep,
                tc.tile_pool(name="msgp2", bufs=1) as mp,
                tc.tile_pool(name="acc2", bufs=2) as ac,
            ):
                for g in range(G):
                    denom = ac.tile([P, 1], F32, tag="denom")
                    out2 = ac.tile([P, O2], F32, tag="out2")
                    first = True
                    for a, k in S.group_chunks[g]:
                        moff = int(S.Doff[g]) + a
                        idxs = ep.tile([P, 8 * CM], I16, tag="idx")
                        nc.sync.dma_start(
                            idxs[:, 0 : 8 * k], idxb[:, 8 * moff : 8 * (moff + k)]
                        )
                        gb = ep.tile([P, CM, C2], F32, tag="gb")
                        for half, s0, n in _chunk_segments(S, g, a, k):
                            src_ap = (
                                fs2[0 : c.ROW_HALF, :]
                                if half == "lo"
                                else fs2[c.ROW_HALF : 2 * c.ROW_HALF, :]
                            )
                            o = s0 - a
                            nc.gpsimd.dma_gather(
                                gb[:, o : o + n, :], src_ap,
                                idxs[:, 8 * o : 8 * (o + n)],
                                P * n, P * n, C2, single_packet=False,
                                queue_num=qctr[0] % 4,
                            )
                            qctr[0] += 1
                        e1 = ep.tile([P, CM], F32, tag="e1")
                        nc.vector.tensor_tensor(
                            out=e1[:, 0:k],
                            in0=gb[:, 0:k, O2],
                            in1=er2tab[:, g : g + 1].to_broadcast([P, k]),
                            op=AL.add,
                        )
                        e2 = ep.tile([P, CM], F32, tag="e2")
                        nc.vector.scalar_tensor_tensor(
                            out=e2[:, 0:k], in0=e1[:, 0:k], scalar=NEG,
                            in1=e1[:, 0:k], op0=AL.mult, op1=AL.max,
                        )
                        e3 = ep.tile([P, CM], F32, tag="e3")
                        nc.scalar.activation(e3[:, 0:k], e2[:, 0:k], AF.Exp)
                        exm = ep.tile([P, CM], F32, tag="exm")
                        nc.vector.tensor_tensor(
                            out=exm[:, 0:k], in0=e3[:, 0:k],
                            in1=mask_sb[:, moff : moff + k], op=AL.mult,
                        )
                        if first:
                            nc.vector.tensor_reduce(
                                out=denom[:, :], in_=exm[:, 0:k], axis=AX.X, op=AL.add
                            )
                        else:
                            dtmp = ep.tile([P, 1], F32, tag="dtmp")
                            nc.vector.tensor_reduce(
                                out=dtmp[:, :], in_=exm[:, 0:k], axis=AX.X, op=AL.add
                            )
                            nc.vector.tensor_tensor(
                                out=denom[:, :], in0=denom[:, :], in1=dtmp[:, :], op=AL.add
                            )
                        msg = mp.tile([P, O2, CM], F32, tag="msg")
                        nc.vector.tensor_tensor(
                            out=msg[:, :, 0:k],
                            in0=gb[:, 0:k, 0:O2].rearrange("p j f -> p f j"),
                            in1=exm[:, None, 0:k].to_broadcast([P, O2, k]),
                            op=AL.mult,
                        )
                        if first:
                            nc.vector.tensor_reduce(
                                out=out2[:, :], in_=msg[:, :, 0:k], axis=AX.X, op=AL.add
                            )
                        else:
                            otmp = ep.tile([P, O2], F32, tag="otmp")
                            nc.vector.tensor_reduce(
                                out=otmp[:, :], in_=msg[:, :, 0:k], axis=AX.X, op=AL.add
                            )
                            nc.vector.tensor_tensor(
                                out=out2[:, :], in0=out2[:, :], in1=otmp[:, :], op=AL.add
                            )
                        first = False
                    if first:
                        nc.vector.memset(denom[:, :], 1.0)
                        nc.vector.memset(out2[:, :], 0.0)
                    rden = ac.tile([P, 1], F32, tag="rden")
                    nc.vector.reciprocal(rden[:, :], denom[:, :])
                    on = ac.tile([P, O2], F32, tag="on")
                    nc.vector.tensor_scalar_mul(on[:], out2[:], rden[:, 0:1])
                    ob = ac.tile([P, O2], F32, tag="ob")
                    nc.vector.tensor_add(ob[:], on[:], b2_sb[:])
                    nc.sync.dma_start(outp[g * P : (g + 1) * P, :], ob[:])
    nc.compile()
    return nc



# ---------------------------------------------------------------- runner with
# persistent executable (avoids re-jit on repeated kernel() calls)
import os as _os
_os.environ.setdefault("JAX_COMPILATION_CACHE_DIR", "/tmp/jax_neff_cache")

N_NODES = 50000
N_EDGES = 800000

_CACHE = {}


class _Runner:
    def __init__(self, cfg, S):
        import jax
        from jax.sharding import Mesh, PartitionSpec
        from jax.experimental.shard_map import shard_map
        import concourse.bass2jax as b2j

        self.cfg = cfg
        nc = build_program(cfg, S)
        b2j.install_neuronx_cc_hook()
        partition_name = (
            nc.partition_id_tensor.name if nc.partition_id_tensor else None
        )
        in_names, out_names, out_avals, zero_outs = [], [], [], []
        for alloc in nc.m.functions[0].allocations:
            if not isinstance(alloc, mybir.MemoryLocationSet):
                continue
            name = alloc.memorylocations[0].name
            if alloc.kind == "ExternalInput":
                if name != partition_name:
                    in_names.append(name)
            elif alloc.kind == "ExternalOutput":
                out_names.append(name)
                shape = tuple(alloc.tensor_shape)
                dtype = mybir.dt.np(alloc.dtype)
                out_avals.append(jax.core.ShapedArray(shape, dtype))
                zero_outs.append(np.zeros(shape, dtype))
        self.n_params = len(in_names)
        self.param_names = list(in_names)
        self.out_names = out_names
        self.zero_outs = zero_outs
        all_in = in_names + out_names
        if partition_name is not None:
            all_in.append(partition_name)

        def _body(*args):
            operands = list(args)
            if partition_name is not None:
                operands.append(b2j.partition_id_tensor())
            outs = b2j._bass_exec_p.bind(
                *operands,
                out_avals=tuple(out_avals),
                in_names=tuple(all_in),
                out_names=tuple(out_names),
                lowering_input_output_aliases=(),
                sim_require_finite=True,
                sim_require_nnan=True,
                nc=nc,
            )
            return tuple(outs)

        devices = jax.devices()[:NC]
        assert len(devices) == NC
        self.mesh = Mesh(np.asarray(devices), ("core",))
        n_io = self.n_params + len(out_names)
        self.fn = jax.jit(
            shard_map(
                _body,
                mesh=self.mesh,
                in_specs=(PartitionSpec("core"),) * n_io,
                out_specs=(PartitionSpec("core"),) * len(out_names),
                check_rep=False,
            ),
            keep_unused=True,
        )
        self.jax = jax
        self.PartitionSpec = PartitionSpec

    def put_inputs(self, in_maps):
        jax = self.jax
        from jax.sharding import NamedSharding

        sh = NamedSharding(self.mesh, self.PartitionSpec("core"))
        args = []
        for name in self.param_names:
            g = np.concatenate([np.asarray(m[name]) for m in in_maps], axis=0)
            args.append(jax.device_put(g, sh))
        for z in self.zero_outs:
            g = np.zeros((NC * z.shape[0], *z.shape[1:]), z.dtype)
            args.append(jax.device_put(g, sh))
        return args

    def __call__(self, args):
        outs = self.fn(*args)
        return [np.asarray(o) for o in outs]


def _get_runner(cfg, S, key):
    r = _CACHE.get(key)
    if r is None:
        r = _Runner(cfg, S)
        _CACHE[key] = r
    return r


def _prepare(inputs):
    cfg = Cfg(N_NODES, N_EDGES)
    x = np.ascontiguousarray(np.asarray(inputs["x"], np.float32))
    src = np.asarray(inputs["src"])
    dst = np.asarray(inputs["dst"])
    S, per_core, orders = preprocess(cfg, x, src, dst)
    shared = make_weights(
        cfg,
        *(np.asarray(inputs[k], np.float32)
          for k in ("W1", "al1", "ar1", "b1", "W2", "al2", "ar2", "b2")),
    )
    in_maps = [dict(shared, **pc) for pc in per_core]
    import hashlib

    key = hashlib.sha1(src.tobytes() + dst.tobytes()).hexdigest()
    return cfg, S, in_maps, orders, key


def kernel(**inputs) -> np.ndarray:
    cfg, S, in_maps, orders, key = _prepare(inputs)
    runner = _get_runner(cfg, S, key)
    args = runner.put_inputs(in_maps)
    outs = runner(args)
    full = outs[runner.out_names.index("out")].reshape(NC, cfg.NPAD, cfg.O2)
    out = np.empty((cfg.N, cfg.O2), np.float32)
    for ci in range(NC):
        out[ci * cfg.NPC + orders[ci]] = full[ci, : cfg.NPC]
    return out
